# revision 6
# baseline (speedup 1.0000x reference)
"""Trainium2 Bass kernel for nn_EnhancedVulnerabilityDetector (moe_routing).

Sharding: data-parallel over batch, 8 cores x 512 tokens each.
Layout: feature-major ([feature_partitions, 512 tokens]) everywhere, so every
linear is a PE matmul with the weight stationary and the token block moving
(N=512, one PSUM bank). Experts are computed densely (all 8) and weighted by
exact top-2 routing weights. Heavy matmuls run in fp32r (fp32 rounded to 11
mantissa bits -> full PE speed); the gate path runs in plain fp32 so the top-2
selection matches the fp32 reference. Routing top-2 itself is done token-major
via PE transposes so reductions are free-axis / per-partition-scalar ops.
"""
import sys
sys.path.insert(0, "/opt/trn_rl_repo")
from contextlib import ExitStack
from functools import partial

import numpy as np

import concourse.tile as tile
from concourse import bacc, mybir
from concourse.bass_utils import run_bass_kernel_spmd

F32 = mybir.dt.float32
F32R = mybir.dt.float32r
AF = mybir.ActivationFunctionType
ALU = mybir.AluOpType

# ---- model dims (hardcoded from the problem spec) ----
G, H, P, F, E, B = 512, 1024, 256, 768, 8, 4096
NCORES = 8
T = B // NCORES          # 512 tokens per core

AST_SPECS = [
    [("lin", G, H), ("ln", H), ("gelu",), ("lin", H, H), ("ln", H), ("gelu",), ("lin", H, P)],
    [("lin", G, H), ("ln", H), ("relu",), ("lin", H, H), ("lin", H, H), ("lin", H, P)],
    [("lin", G, H), ("ln", H), ("elu",), ("lin", H, H), ("tanh",), ("lin", H, P)],
    [("lin", G, H // 2), ("ln", H // 2), ("gelu",), ("lin", H // 2, P)],
]
PDG_SPECS = [
    [("lin", G, H), ("ln", H), ("gelu",), ("lin", H, H), ("ln", H), ("gelu",), ("lin", H, H), ("gelu",), ("lin", H, P)],
    [("lin", G, H), ("ln", H), ("sigmoid",), ("lin", H, H), ("relu",), ("lin", H, P)],
    [("lin", G, 2 * H), ("ln", 2 * H), ("relu",), ("lin", 2 * H, H), ("ln", H), ("gelu",), ("lin", H, P)],
    [("lin", G, H // 2), ("ln", H // 2), ("leaky",), ("lin", H // 2, P)],
]
CFG_SPECS = [
    [("lin", G, H), ("ln", H), ("gelu",), ("lin", H, H), ("ln", H), ("gelu",), ("lin", H, H), ("ln", H), ("gelu",), ("lin", H, P)],
    [("lin", G, H), ("ln", H), ("tanh",), ("lin", H, H), ("ln", H), ("relu",), ("lin", H, P)],
    [("lin", G, H), ("ln", H), ("gelu",), ("lin", H, 2 * H), ("sigmoid",), ("lin", 2 * H, H), ("ln", H), ("gelu",), ("lin", H, P)],
    [("lin", G, H // 2), ("relu",), ("lin", H // 2, P)],
]
GATE_SPEC = [("lin", G, H // 2), ("ln", H // 2), ("gelu",), ("lin", H // 2, E)]
INTER_SPEC = [("lin", F, F), ("ln", F), ("gelu",), ("lin", F, F), ("ln", F), ("sigmoid",)]
OUT_SPEC = [("lin", F, F), ("ln", F), ("gelu",), ("lin", F, F)]
MODS = ("ast", "pdg", "cfg")
MOD_SPECS = {"ast": AST_SPECS, "pdg": PDG_SPECS, "cfg": CFG_SPECS}
ACT_FUNCS = {"gelu": AF.Gelu, "relu": AF.Relu, "tanh": AF.Tanh,
             "sigmoid": AF.Sigmoid, "leaky": AF.Lrelu}
ACT_NAMES = set(ACT_FUNCS) | {"elu"}


def round_f32r(a):
    b = np.ascontiguousarray(np.asarray(a, np.float32)).view(np.uint32)
    r = (b + 0x7FF + ((b >> 12) & 1)) & np.uint32(0xFFFFF000)
    return r.view(np.float32).copy()


def lin_names(mod):
    out = []
    for e in range(E):
        li = 0
        for op in MOD_SPECS[mod][e % 4]:
            if op[0] == "lin":
                out.append((f"{mod}_e{e}_w{li}", op[1], op[2]))
                li += 1
    return out


class Prog:
    pass


# --------------------------------------------------------------------------
# emitters
# --------------------------------------------------------------------------

def emit_linear(pg, x_wide, fi, fo, w_ap, dt, consume):
    """y = W.T @ x feature-major. x_wide: [128, (fi/128)*T]. For each output
    block co (width mw<=128) calls consume(psum_tile, co, mw)."""
    nc = pg.nc
    Cin = fi // 128
    Cout = (fo + 127) // 128
    for og0 in range(0, Cout, 4):
        og1 = min(og0 + 4, Cout)
        blocks = []
        for co in range(og0, og1):
            mw = min(128, fo - co * 128)
            blocks.append((co, mw, pg.mmps.tile([mw, T], F32, tag="mm", name="mmblk")))
        f0 = og0 * 128
        f1 = min(og1 * 128, fo)
        for ci in range(Cin):
            wkb = pg.wpool.tile([128, f1 - f0], dt, tag="wkb")
            nc.sync.dma_start(wkb[:, :], w_ap[ci * 128:(ci + 1) * 128, f0:f1])
            for co, mw, ps in blocks:
                nc.tensor.matmul(
                    ps[:], wkb[:, co * 128 - f0:co * 128 - f0 + mw],
                    x_wide[:, ci * T:(ci + 1) * T],
                    start=(ci == 0), stop=(ci == Cin - 1))
        for co, mw, ps in blocks:
            consume(ps, co, mw)


def emit_act(pg, dst, src, act, extra_scale=None):
    """dst = act(src) on the scalar engine (src may be PSUM)."""
    nc = pg.nc
    kw = {}
    if extra_scale is not None:
        kw["scale"] = extra_scale
    if act is None:
        nc.scalar.activation(dst, src, AF.Copy, **kw)
    elif act == "elu":
        emit_elu(pg, src, dst)
    elif act == "leaky":
        nc.scalar.activation(dst, src, AF.Lrelu, alpha=0.01, **kw)
    else:
        nc.scalar.activation(dst, src, ACT_FUNCS[act], **kw)


def emit_elu(pg, src, dst):
    """dst = elu(src) = relu(src) + exp(min(src,0)) - 1."""
    nc = pg.nc
    t1 = pg.spool.tile([128, T], F32, tag="elu1")
    nc.vector.tensor_scalar(t1[:], src, 0.0, None, ALU.min)
    t2 = pg.spool.tile([128, T], F32, tag="elu2")
    nc.scalar.activation(t2[:], t1[:], AF.Exp)
    t3 = pg.spool.tile([128, T], F32, tag="elu3")
    nc.scalar.activation(t3[:], src, AF.Relu)
    nc.vector.tensor_tensor(t2[:], t2[:], t3[:], ALU.add)
    nc.vector.tensor_scalar(dst, t2[:], 1.0, None, ALU.subtract)


def emit_ln_act(pg, x_wide, Cf, act, rdt):
    """In-place LayerNorm over Cf*128 features + activation on x_wide."""
    nc = pg.nc
    ones_col = pg.C["ones_col_r" if rdt == F32R else "ones_col_f"]
    ones_row = pg.C["ones_row_r" if rdt == F32R else "ones_row_f"]
    D = float(Cf * 128)
    s1 = pg.stps.tile([1, T], F32, tag="s1")
    s2 = pg.stps.tile([1, T], F32, tag="s2")
    for c in range(Cf):
        xb = x_wide[:, c * T:(c + 1) * T]
        nc.tensor.matmul(s1[:], ones_col[:], xb, start=(c == 0), stop=(c == Cf - 1))
        sq = pg.spool.tile([128, T], rdt, tag="sq")
        nc.vector.tensor_tensor(sq[:], xb, xb, ALU.mult)
        nc.tensor.matmul(s2[:], ones_col[:], sq[:], start=(c == 0), stop=(c == Cf - 1))
    mu = pg.rpool.tile([1, T], F32, tag="mu")
    nc.vector.tensor_scalar(mu[:], s1[:], 1.0 / D, None, ALU.mult)
    trow = pg.rpool.tile([1, T], F32, tag="trow")
    nc.vector.tensor_tensor(trow[:], mu[:], mu[:], ALU.mult)
    nc.vector.scalar_tensor_tensor(trow[:], s2[:], 1.0 / D, trow[:], ALU.mult, ALU.subtract)
    nc.scalar.activation(trow[:], trow[:], AF.Sqrt, bias=pg.eps[:])
    nc.vector.reciprocal(trow[:], trow[:])
    inv_r = pg.rpool.tile([1, T], rdt, tag="inv_r")
    nc.scalar.copy(inv_r[:], trow[:])
    nmu_r = pg.rpool.tile([1, T], rdt, tag="nmu_r")
    nc.vector.scalar_tensor_tensor(nmu_r[:], mu[:], -1.0, trow[:], ALU.mult, ALU.mult)
    invb = pg.stps.tile([128, T], F32, tag="invb")
    nc.tensor.matmul(invb[:], ones_row[0:1, :], inv_r[:], start=True, stop=True)
    nmub = pg.stps.tile([128, T], F32, tag="nmub")
    nc.tensor.matmul(nmub[:], ones_row[0:1, :], nmu_r[:], start=True, stop=True)
    for c in range(Cf):
        xb = x_wide[:, c * T:(c + 1) * T]
        tmp = pg.spool.tile([128, T], F32, tag="lntmp")
        nc.vector.tensor_tensor(tmp[:], xb, invb[:], ALU.mult)
        nc.vector.tensor_tensor(tmp[:], tmp[:], nmub[:], ALU.add)
        emit_act(pg, xb, tmp[:], act)   # in-place write back into x_wide


def htile(pg, Cout, dt):
    pool = pg.p1a if Cout >= 16 or Cout <= 4 else pg.p1b
    return pool.tile([128, Cout * T], dt, tag=f"h{Cout}", name=f"h{Cout}")


def emit_chain(pg, x_wide, spec, w_aps, dt, final_consume=None):
    """Run a lin/ln/act spec chain feature-major. Returns the final wide tile,
    or None if the chain ends with a bare lin consumed by final_consume."""
    nc = pg.nc
    h = x_wide
    wi = 0
    i = 0
    n = len(spec)
    while i < n:
        op = spec[i]
        assert op[0] == "lin", op
        fi, fo = op[1], op[2]
        if i == n - 1:  # final linear
            emit_linear(pg, h, fi, fo, w_aps[wi], dt, final_consume)
            return None
        nxt = spec[i + 1][0]
        nxt2 = spec[i + 2][0] if i + 2 < n else None
        Cout = fo // 128
        hn = htile(pg, Cout, dt)
        if nxt == "ln":
            act = nxt2 if nxt2 in ACT_NAMES else None

            def cp(ps, co, mw, hn=hn):
                nc.scalar.copy(hn[:, co * T:(co + 1) * T], ps[:])
            emit_linear(pg, h, fi, fo, w_aps[wi], dt, cp)
            emit_ln_act(pg, hn, Cout, act, dt)
            i += 3 if act else 2
        elif nxt in ACT_NAMES:
            def ac(ps, co, mw, hn=hn, nxt=nxt):
                emit_act(pg, hn[:, co * T:(co + 1) * T], ps[:], nxt)
            emit_linear(pg, h, fi, fo, w_aps[wi], dt, ac)
            i += 2
        else:  # lin follows lin directly
            def cc(ps, co, mw, hn=hn):
                nc.scalar.copy(hn[:, co * T:(co + 1) * T], ps[:])
            emit_linear(pg, h, fi, fo, w_aps[wi], dt, cc)
            i += 1
        h = hn
        wi += 1
    return h


def emit_routing(pg, logits_sb):
    """logits_sb [E, T] f32 -> top-2 softmax routing weights [E, T] f32r."""
    nc = pg.nc
    ident = pg.C["ident"]
    wps = pg.stps.tile([E, T], F32, tag="invb")
    for c in range(T // 128):
        tp = pg.mmps.tile([128, E], F32, tag="mm")
        nc.tensor.transpose(tp[:], logits_sb[:, c * 128:(c + 1) * 128], ident[0:E, 0:E])
        ltok = pg.rpool.tile([128, E], F32, tag="ltok")
        nc.scalar.copy(ltok[:], tp[:])
        v1 = pg.rpool.tile([128, 1], F32, tag="rt_v1")
        nc.vector.tensor_reduce(v1[:], ltok[:], mybir.AxisListType.X, ALU.max)
        eq1 = pg.rpool.tile([128, E], F32, tag="rt_eq1")
        nc.vector.tensor_scalar(eq1[:], ltok[:], v1[:], None, ALU.is_equal)
        msk = pg.rpool.tile([128, E], F32, tag="rt_msk")
        nc.vector.scalar_tensor_tensor(msk[:], eq1[:], -1e30, ltok[:], ALU.mult, ALU.add)
        v2 = pg.rpool.tile([128, 1], F32, tag="rt_v2")
        nc.vector.tensor_reduce(v2[:], msk[:], mybir.AxisListType.X, ALU.max)
        eq2 = pg.rpool.tile([128, E], F32, tag="rt_eq2")
        nc.vector.tensor_scalar(eq2[:], msk[:], v2[:], None, ALU.is_equal)
        d = pg.rpool.tile([128, 1], F32, tag="rt_d")
        nc.vector.tensor_tensor(d[:], v2[:], v1[:], ALU.subtract)
        ex = pg.rpool.tile([128, 1], F32, tag="rt_ex")
        nc.scalar.activation(ex[:], d[:], AF.Exp)
        den = pg.rpool.tile([128, 1], F32, tag="rt_den")
        nc.vector.tensor_scalar(den[:], ex[:], 1.0, None, ALU.add)
        w1 = pg.rpool.tile([128, 1], F32, tag="rt_w1")
        nc.vector.reciprocal(w1[:], den[:])
        w2 = pg.rpool.tile([128, 1], F32, tag="rt_w2")
        nc.vector.tensor_tensor(w2[:], ex[:], w1[:], ALU.mult)
        wt = pg.rpool.tile([128, E], F32, tag="rt_wt")
        nc.vector.tensor_scalar(wt[:], eq1[:], w1[:], None, ALU.mult)
        wt2 = pg.rpool.tile([128, E], F32, tag="rt_wt2")
        nc.vector.tensor_scalar(wt2[:], eq2[:], w2[:], None, ALU.mult)
        wt3 = pg.rpool.tile([128, E], F32, tag="rt_wt3")
        nc.vector.tensor_tensor(wt3[:], wt[:], wt2[:], ALU.add)
        nc.tensor.transpose(wps[0:E, c * 128:(c + 1) * 128], wt3[:], ident[:])
    wf = pg.p1b.tile([E, T], F32R, tag="wfull")
    nc.scalar.copy(wf[:], wps[:])
    return wf


def emit_modality(pg, dram, m, moe_out):
    """Gate + routing + dense experts for one modality. Writes moe_out
    ([128, 2T] f32r = the weighted top-2 mixture, feature-major P=256)."""
    nc = pg.nc
    ident = pg.C["ident"]

    # load + transpose input embeddings to feature-major, f32 and f32r copies
    xT_f = pg.p1a.tile([128, (G // 128) * T], F32, tag="xT_f")
    xT_r = pg.p1a.tile([128, (G // 128) * T], F32R, tag="xT_r")
    for tc4 in range(T // 128):
        xin = pg.spool.tile([128, G], F32, tag="xin")
        nc.sync.dma_start(xin[:], dram[f"x_{m}"][tc4 * 128:(tc4 + 1) * 128, :])
        for fb in range(G // 128):
            tp = pg.mmps.tile([128, 128], F32, tag="mm")
            nc.tensor.transpose(tp[:], xin[:, fb * 128:(fb + 1) * 128], ident[:])
            dst = slice(fb * T + tc4 * 128, fb * T + (tc4 + 1) * 128)
            nc.scalar.copy(xT_f[:, dst], tp[:])
            nc.vector.tensor_copy(xT_r[:, dst], tp[:])

    # gate (fp32) -> logits [E, T]
    logits_sb = pg.p1b.tile([E, T], F32, tag="logits")

    def fc_logits(ps, co, mw):
        nc.scalar.copy(logits_sb[:], ps[:])
    emit_chain(pg, xT_f, GATE_SPEC,
               [dram[f"{m}_g_w0"], dram[f"{m}_g_w1"]], F32, fc_logits)
    wf = emit_routing(pg, logits_sb)

    # dense experts, weighted accumulation into macc
    macc = pg.p1a.tile([128, 2 * T], F32, tag="macc")
    w_iter = iter(lin_names(m))
    for e in range(E):
        spec = MOD_SPECS[m][e % 4]
        w_aps = [dram[next(w_iter)[0]] for op in spec if op[0] == "lin"]
        wb_ps = pg.mmps.tile([128, T], F32, tag="mm")
        nc.tensor.matmul(wb_ps[:], pg.C["esel"][:, e * 128:(e + 1) * 128], wf[:],
                         start=True, stop=True)
        wb = pg.spool.tile([128, T], F32, tag="wb")
        nc.scalar.copy(wb[:], wb_ps[:])

        def fc(ps, co, mw, e=e, wb=wb):
            blk = macc[:, co * T:(co + 1) * T]
            if e == 0:
                nc.vector.scalar_tensor_tensor(blk, ps[:], 1.0, wb[:],
                                               ALU.mult, ALU.mult)
            else:
                tw = pg.spool.tile([128, T], F32, tag="tw")
                nc.vector.scalar_tensor_tensor(tw[:], ps[:], 1.0, wb[:],
                                               ALU.mult, ALU.mult)
                nc.vector.tensor_tensor(blk, blk, tw[:], ALU.add)
        emit_chain(pg, xT_r, spec, w_aps, F32R, fc)
    nc.scalar.copy(moe_out[:], macc[:])


def emit_attention_and_head(pg, dram, moe_r, out_ap):
    nc = pg.nc
    ident = pg.C["ident"]

    # qkv per position: [128, 6T] f32r (q=blocks 0-1, k=2-3, v=4-5)
    qkv = []
    for pi, m in enumerate(MODS):
        qt = pg.p1a.tile([128, 6 * T], F32R, tag=f"qkv{pi}")

        def cq(ps, co, mw, qt=qt):
            nc.scalar.copy(qt[:, co * T:(co + 1) * T], ps[:])
        emit_linear(pg, moe_r[m], P, 3 * P, dram["attn_w_in"], F32R, cq)
        qkv.append(qt)

    combined = pg.p1a.tile([128, 6 * T], F32R, tag="comb")
    sc_tags = ("s1", "s2", "nmub")
    for qp in range(3):
        # scores [4, T] per kp  (scaled by 1/sqrt(HD)=0.125)
        scs = []
        for kp in range(3):
            scp = pg.stps.tile([4, T], F32, tag=sc_tags[kp])
            for c in range(2):
                prod = pg.spool.tile([128, T], F32R, tag="prod")
                nc.vector.tensor_tensor(prod[:], qkv[qp][:, c * T:(c + 1) * T],
                                        qkv[kp][:, (2 + c) * T:(3 + c) * T], ALU.mult)
                nc.tensor.matmul(scp[:], pg.C[f"ind{c}"][:], prod[:],
                                 start=(c == 0), stop=(c == 1))
            sc = pg.rpool.tile([4, T], F32, tag=f"sc{kp}")
            nc.scalar.activation(sc[:], scp[:], AF.Copy, scale=0.125)
            scs.append(sc)
        # softmax over kp
        mx = pg.rpool.tile([4, T], F32, tag="mx")
        nc.vector.tensor_tensor(mx[:], scs[0][:], scs[1][:], ALU.max)
        nc.vector.tensor_tensor(mx[:], mx[:], scs[2][:], ALU.max)
        es = []
        for kp in range(3):
            ek = scs[kp]
            nc.vector.tensor_tensor(ek[:], ek[:], mx[:], ALU.subtract)
            nc.scalar.activation(ek[:], ek[:], AF.Exp)
            es.append(ek)
        den = pg.rpool.tile([4, T], F32, tag="den4")
        nc.vector.tensor_tensor(den[:], es[0][:], es[1][:], ALU.add)
        nc.vector.tensor_tensor(den[:], den[:], es[2][:], ALU.add)
        rec = den
        nc.vector.reciprocal(rec[:], den[:])
        # out: o = sum_kp bcast(att_kp) * v_kp   -> o_r [128, 2T] f32r
        oacc = pg.p1b.tile([128, 2 * T], F32, tag="oacc")
        o_r = pg.p1b.tile([128, 2 * T], F32R, tag="o_r")
        for kp in range(3):
            att = pg.rpool.tile([4, T], F32R, tag="attk")
            nc.vector.tensor_tensor(att[:], es[kp][:], rec[:], ALU.mult)
            for c in range(2):
                bc = pg.mmps.tile([128, T], F32, tag="mm")
                nc.tensor.matmul(bc[:], pg.C[f"bind{c}"][:], att[:],
                                 start=True, stop=True)
                vblk = qkv[kp][:, (4 + c) * T:(5 + c) * T]
                if kp == 0:
                    nc.vector.scalar_tensor_tensor(
                        oacc[:, c * T:(c + 1) * T], vblk, 1.0, bc[:], ALU.mult, ALU.mult)
                elif kp == 1:
                    tv = pg.spool.tile([128, T], F32, tag="tv")
                    nc.vector.scalar_tensor_tensor(tv[:], vblk, 1.0, bc[:],
                                                   ALU.mult, ALU.mult)
                    nc.vector.tensor_tensor(oacc[:, c * T:(c + 1) * T],
                                            oacc[:, c * T:(c + 1) * T], tv[:], ALU.add)
                else:
                    tv = pg.spool.tile([128, T], F32, tag="tv")
                    nc.vector.scalar_tensor_tensor(tv[:], vblk, 1.0, bc[:],
                                                   ALU.mult, ALU.mult)
                    nc.vector.tensor_tensor(o_r[:, c * T:(c + 1) * T],
                                            oacc[:, c * T:(c + 1) * T], tv[:], ALU.add)

        def co_out(ps, co, mw, qp=qp):
            nc.scalar.copy(combined[:, (qp * 2 + co) * T:(qp * 2 + co + 1) * T], ps[:])
        emit_linear(pg, o_r, P, P, dram["attn_w_out"], F32R, co_out)

    # inter MLP -> sigmoid gate; combined *= gate
    gate6 = emit_chain(pg, combined, INTER_SPEC,
                       [dram["inter_w0"], dram["inter_w1"]], F32R)
    for c in range(6):
        nc.vector.tensor_tensor(combined[:, c * T:(c + 1) * T],
                                combined[:, c * T:(c + 1) * T],
                                gate6[:, c * T:(c + 1) * T], ALU.mult)

    # out MLP -> final [F, T] then transpose to [T, F] and DMA out
    fin = pg.p1a.tile([128, 6 * T], F32, tag="fin")

    def fc_fin(ps, co, mw):
        nc.scalar.copy(fin[:, co * T:(co + 1) * T], ps[:])
    emit_chain(pg, combined, OUT_SPEC, [dram["out_w0"], dram["out_w1"]], F32R, fc_fin)

    for tc4 in range(T // 128):
        ot = pg.p1b.tile([128, F], F32, tag="otok")
        for fb in range(F // 128):
            tp = pg.mmps.tile([128, 128], F32, tag="mm")
            nc.tensor.transpose(
                tp[:], fin[:, fb * T + tc4 * 128:fb * T + (tc4 + 1) * 128], ident[:])
            nc.vector.tensor_copy(ot[:, fb * 128:(fb + 1) * 128], tp[:])
        nc.sync.dma_start(out_ap[tc4 * 128:(tc4 + 1) * 128, :], ot[:])


def emit_all(pg, dram, out_ap):
    nc, tc, ctx = pg.nc, pg.tc, pg.ctx
    cpool = ctx.enter_context(tc.tile_pool(name="consts", bufs=1))
    gpool = ctx.enter_context(tc.tile_pool(name="glob", bufs=1))
    pg.wpool = ctx.enter_context(tc.tile_pool(name="wkb", bufs=3))
    pg.spool = ctx.enter_context(tc.tile_pool(name="small", bufs=2))
    pg.rpool = ctx.enter_context(tc.tile_pool(name="rows", bufs=1))
    pg.mmps = ctx.enter_context(tc.tile_pool(name="mmps", bufs=4, space="PSUM"))
    pg.stps = ctx.enter_context(tc.tile_pool(name="stps", bufs=1, space="PSUM"))

    pg.C = {}
    for nm in ("ident", "ones_col_f", "ones_col_r", "ones_row_f", "ones_row_r",
               "esel", "ind0", "ind1", "bind0", "bind1"):
        t = cpool.tile(list(dram[nm].shape), dram[nm].dtype, tag=nm, name=f"c_{nm}")
        nc.sync.dma_start(t[:], dram[nm][:])
        pg.C[nm] = t
    pg.eps = cpool.tile([1, 1], F32, tag="eps")
    nc.vector.memset(pg.eps[:], 1e-5)

    moe_r = {}
    for m in MODS:
        moe_r[m] = gpool.tile([128, 2 * T], F32R, tag=f"moe_{m}", name=f"moe_{m}")

    with tc.tile_pool(name="ph1a", bufs=1) as p1a, \
         tc.tile_pool(name="ph1b", bufs=2) as p1b:
        pg.p1a, pg.p1b = p1a, p1b
        for m in MODS:
            emit_modality(pg, dram, m, moe_r[m])

    with tc.tile_pool(name="ph2a", bufs=1) as p2a, \
         tc.tile_pool(name="ph2b", bufs=2) as p2b:
        pg.p1a, pg.p1b = p2a, p2b
        emit_attention_and_head(pg, dram, moe_r, out_ap)


def build_program():
    nc = bacc.Bacc("TRN2", target_bir_lowering=False, debug=False,
                   num_devices=NCORES)
    dram = {}

    def din(name, shape, dt):
        dram[name] = nc.dram_tensor(name, list(shape), dt, kind="ExternalInput").ap()

    for m in MODS:
        din(f"x_{m}", (T, G), F32)
        for nm, fi, fo in lin_names(m):
            din(nm, (fi, fo), F32R)
        din(f"{m}_g_w0", (G, H // 2), F32)
        din(f"{m}_g_w1", (H // 2, E), F32)
    din("attn_w_in", (P, 3 * P), F32R)
    din("attn_w_out", (P, P), F32R)
    for i in range(2):
        din(f"inter_w{i}", (F, F), F32R)
        din(f"out_w{i}", (F, F), F32R)
    din("ident", (128, 128), F32)
    din("ones_col_f", (128, 1), F32)
    din("ones_col_r", (128, 1), F32R)
    din("ones_row_f", (1, 128), F32)
    din("ones_row_r", (1, 128), F32R)
    din("esel", (E, E * 128), F32R)
    din("ind0", (128, 4), F32R)
    din("ind1", (128, 4), F32R)
    din("bind0", (4, 128), F32R)
    din("bind1", (4, 128), F32R)
    out_ap = nc.dram_tensor("out", [T, F], F32, kind="ExternalOutput").ap()

    pg = Prog()
    with tile.TileContext(nc) as tc, ExitStack() as ctx:
        pg.nc, pg.tc, pg.ctx = nc, tc, ctx
        emit_all(pg, dram, out_ap)
    nc.compile()
    return nc


# --------------------------------------------------------------------------
# host side
# --------------------------------------------------------------------------

_NC = None
_TRACE = False
_LAST_EXEC_NS = None


def _consts():
    c = {}
    c["ident"] = np.eye(128, dtype=np.float32)
    c["ones_col_f"] = np.ones((128, 1), np.float32)
    c["ones_col_r"] = np.ones((128, 1), np.float32)
    c["ones_row_f"] = np.ones((1, 128), np.float32)
    c["ones_row_r"] = np.ones((1, 128), np.float32)
    esel = np.zeros((E, E * 128), np.float32)
    for e in range(E):
        esel[e, e * 128:(e + 1) * 128] = 1.0
    c["esel"] = esel
    for ci in range(2):
        ind = np.zeros((128, 4), np.float32)
        ind[0:64, 2 * ci] = 1.0
        ind[64:128, 2 * ci + 1] = 1.0
        c[f"ind{ci}"] = ind
        c[f"bind{ci}"] = ind.T.copy()
    return c


def _flatten_params(params):
    """-> dict of np arrays keyed by dram tensor names; asserts the zero/unit
    structure of biases and LN params this kernel relies on."""
    out = {}

    def getw(plist, spec, prefix):
        li = 0
        pi = 0
        for op in spec:
            if op[0] == "lin":
                w, b = plist[pi]
                assert not np.any(np.asarray(b)), f"nonzero bias at {prefix}_w{li}"
                out[f"{prefix}_w{li}"] = np.asarray(w, np.float32)
                li += 1
                pi += 1
            elif op[0] == "ln":
                g, b = plist[pi]
                assert np.all(np.asarray(g) == 1.0) and not np.any(np.asarray(b)), \
                    f"non-identity LN at {prefix}"
                pi += 1

    for m in MODS:
        for e in range(E):
            getw(params[m][e], MOD_SPECS[m][e % 4], f"{m}_e{e}")
        gl = params[f"{m}_gate"]
        getw(gl, GATE_SPEC, f"{m}_g")
    ap = params["attn"]
    assert not np.any(np.asarray(ap["b_in"])) and not np.any(np.asarray(ap["b_out"]))
    out["attn_w_in"] = np.asarray(ap["w_in"], np.float32)
    out["attn_w_out"] = np.asarray(ap["w_out"], np.float32)
    getw(params["inter"], INTER_SPEC, "inter")
    getw(params["out"], OUT_SPEC, "out")
    return out


def kernel(ast_emb, pdg_emb, cfg_emb, params):
    global _NC, _LAST_EXEC_NS
    if _NC is None:
        _NC = build_program()
    flat = _flatten_params(params)
    # round the f32r weights on host (HW truncates to 20 bits on read)
    shared = {}
    f32_keys = {f"{m}_g_w{i}" for m in MODS for i in range(2)}
    for k, v in flat.items():
        shared[k] = v if k in f32_keys else round_f32r(v)
    shared.update(_consts())

    embs = {"ast": np.asarray(ast_emb, np.float32),
            "pdg": np.asarray(pdg_emb, np.float32),
            "cfg": np.asarray(cfg_emb, np.float32)}
    in_maps = []
    for core in range(NCORES):
        im = dict(shared)
        for m in MODS:
            im[f"x_{m}"] = embs[m][core * T:(core + 1) * T]
        in_maps.append(im)

    res = run_bass_kernel_spmd(_NC, in_maps, core_ids=list(range(NCORES)),
                               trace=_TRACE)
    _LAST_EXEC_NS = res.exec_time_ns
    return np.concatenate([res.results[c]["out"] for c in range(NCORES)], axis=0)


# revision 8
# speedup vs baseline: 1.2983x; 1.2983x over previous
"""Trainium2 Bass kernel for nn_EnhancedVulnerabilityDetector (moe_routing).

Sharding: data-parallel over batch, 8 cores x 512 tokens each.
Layout: feature-major ([feature_partitions, 512 tokens]) everywhere, so every
linear is a PE matmul with the weight stationary and the token block moving
(N=512, one PSUM bank). Experts are computed densely (all 8) and weighted by
exact top-2 routing weights. Heavy matmuls run in fp32r (fp32 rounded to 11
mantissa bits -> full PE speed); the gate path runs in plain fp32 so the top-2
selection matches the fp32 reference. Routing top-2 itself is done token-major
via PE transposes so reductions are free-axis / per-partition-scalar ops.
"""
import sys
sys.path.insert(0, "/opt/trn_rl_repo")
from contextlib import ExitStack
from functools import partial

import numpy as np

import concourse.tile as tile
from concourse import bacc, mybir
from concourse.bass_utils import run_bass_kernel_spmd

F32 = mybir.dt.float32
F32R = mybir.dt.float32r
AF = mybir.ActivationFunctionType
ALU = mybir.AluOpType

# ---- model dims (hardcoded from the problem spec) ----
G, H, P, F, E, B = 512, 1024, 256, 768, 8, 4096
NCORES = 8
T = B // NCORES          # 512 tokens per core

AST_SPECS = [
    [("lin", G, H), ("ln", H), ("gelu",), ("lin", H, H), ("ln", H), ("gelu",), ("lin", H, P)],
    [("lin", G, H), ("ln", H), ("relu",), ("lin", H, H), ("lin", H, H), ("lin", H, P)],
    [("lin", G, H), ("ln", H), ("elu",), ("lin", H, H), ("tanh",), ("lin", H, P)],
    [("lin", G, H // 2), ("ln", H // 2), ("gelu",), ("lin", H // 2, P)],
]
PDG_SPECS = [
    [("lin", G, H), ("ln", H), ("gelu",), ("lin", H, H), ("ln", H), ("gelu",), ("lin", H, H), ("gelu",), ("lin", H, P)],
    [("lin", G, H), ("ln", H), ("sigmoid",), ("lin", H, H), ("relu",), ("lin", H, P)],
    [("lin", G, 2 * H), ("ln", 2 * H), ("relu",), ("lin", 2 * H, H), ("ln", H), ("gelu",), ("lin", H, P)],
    [("lin", G, H // 2), ("ln", H // 2), ("leaky",), ("lin", H // 2, P)],
]
CFG_SPECS = [
    [("lin", G, H), ("ln", H), ("gelu",), ("lin", H, H), ("ln", H), ("gelu",), ("lin", H, H), ("ln", H), ("gelu",), ("lin", H, P)],
    [("lin", G, H), ("ln", H), ("tanh",), ("lin", H, H), ("ln", H), ("relu",), ("lin", H, P)],
    [("lin", G, H), ("ln", H), ("gelu",), ("lin", H, 2 * H), ("sigmoid",), ("lin", 2 * H, H), ("ln", H), ("gelu",), ("lin", H, P)],
    [("lin", G, H // 2), ("relu",), ("lin", H // 2, P)],
]
GATE_SPEC = [("lin", G, H // 2), ("ln", H // 2), ("gelu",), ("lin", H // 2, E)]
INTER_SPEC = [("lin", F, F), ("ln", F), ("gelu",), ("lin", F, F), ("ln", F), ("sigmoid",)]
OUT_SPEC = [("lin", F, F), ("ln", F), ("gelu",), ("lin", F, F)]
MODS = ("ast", "pdg", "cfg")
MOD_SPECS = {"ast": AST_SPECS, "pdg": PDG_SPECS, "cfg": CFG_SPECS}
ACT_FUNCS = {"gelu": AF.Gelu, "relu": AF.Relu, "tanh": AF.Tanh,
             "sigmoid": AF.Sigmoid, "leaky": AF.Lrelu}
ACT_NAMES = set(ACT_FUNCS) | {"elu"}


def round_f32r(a):
    b = np.ascontiguousarray(np.asarray(a, np.float32)).view(np.uint32)
    r = (b + 0x7FF + ((b >> 12) & 1)) & np.uint32(0xFFFFF000)
    return r.view(np.float32).copy()


def lin_names(mod):
    out = []
    for e in range(E):
        li = 0
        for op in MOD_SPECS[mod][e % 4]:
            if op[0] == "lin":
                out.append((f"{mod}_e{e}_w{li}", op[1], op[2]))
                li += 1
    return out


class Prog:
    pass


# --------------------------------------------------------------------------
# emitters
# --------------------------------------------------------------------------

def emit_linear(pg, x_wide, fi, fo, w_ap, dt, consume):
    """y = W.T @ x feature-major. x_wide: [128, (fi/128)*T]. For each output
    block co (width mw<=128) calls consume(psum_tile, co, mw)."""
    nc = pg.nc
    Cin = fi // 128
    Cout = (fo + 127) // 128
    for og0 in range(0, Cout, 4):
        og1 = min(og0 + 4, Cout)
        blocks = []
        for co in range(og0, og1):
            mw = min(128, fo - co * 128)
            blocks.append((co, mw, pg.mmps.tile([mw, T], F32, tag="mm", name="mmblk")))
        f0 = og0 * 128
        f1 = min(og1 * 128, fo)
        for ci in range(Cin):
            wkb = pg.wpool.tile([128, f1 - f0], dt, tag="wkb")
            nc.sync.dma_start(wkb[:, :], w_ap[ci * 128:(ci + 1) * 128, f0:f1])
            for co, mw, ps in blocks:
                nc.tensor.matmul(
                    ps[:], wkb[:, co * 128 - f0:co * 128 - f0 + mw],
                    x_wide[:, ci * T:(ci + 1) * T],
                    start=(ci == 0), stop=(ci == Cin - 1))
        for co, mw, ps in blocks:
            consume(ps, co, mw)


def emit_act(pg, dst, src, act, extra_scale=None):
    """dst = act(src) on the scalar engine (src may be PSUM)."""
    nc = pg.nc
    kw = {}
    if extra_scale is not None:
        kw["scale"] = extra_scale
    if act is None:
        nc.scalar.activation(dst, src, AF.Copy, **kw)
    elif act == "elu":
        emit_elu(pg, src, dst)
    elif act == "leaky":
        nc.scalar.activation(dst, src, AF.Lrelu, alpha=0.01, **kw)
    else:
        nc.scalar.activation(dst, src, ACT_FUNCS[act], **kw)


def emit_elu(pg, src, dst):
    """dst = elu(src) = relu(src) + exp(min(src,0)) - 1."""
    nc = pg.nc
    t1 = pg.spool.tile([128, T], F32, tag="elu1")
    nc.vector.tensor_scalar(t1[:], src, 0.0, None, ALU.min)
    t2 = pg.spool.tile([128, T], F32, tag="elu2")
    nc.scalar.activation(t2[:], t1[:], AF.Exp)
    t3 = pg.spool.tile([128, T], F32, tag="elu3")
    nc.scalar.activation(t3[:], src, AF.Relu)
    nc.vector.tensor_tensor(t2[:], t2[:], t3[:], ALU.add)
    nc.vector.tensor_scalar(dst, t2[:], 1.0, None, ALU.subtract)


def emit_ln_act(pg, x_wide, Cf, act, rdt):
    """In-place LayerNorm over Cf*128 features + activation on x_wide."""
    nc = pg.nc
    ones_col = pg.C["ones_col_r" if rdt == F32R else "ones_col_f"]
    ones_row = pg.C["ones_row_r" if rdt == F32R else "ones_row_f"]
    D = float(Cf * 128)
    s1 = pg.stps.tile([1, T], F32, tag="s1")
    s2 = pg.stps.tile([1, T], F32, tag="s2")
    for c in range(Cf):
        xb = x_wide[:, c * T:(c + 1) * T]
        nc.tensor.matmul(s1[:], ones_col[:], xb, start=(c == 0), stop=(c == Cf - 1))
        sq = pg.spool.tile([128, T], rdt, tag="sq")
        nc.vector.tensor_tensor(sq[:], xb, xb, ALU.mult)
        nc.tensor.matmul(s2[:], ones_col[:], sq[:], start=(c == 0), stop=(c == Cf - 1))
    mu = pg.rpool.tile([1, T], F32, tag="mu")
    nc.vector.tensor_scalar(mu[:], s1[:], 1.0 / D, None, ALU.mult)
    trow = pg.rpool.tile([1, T], F32, tag="trow")
    nc.vector.tensor_tensor(trow[:], mu[:], mu[:], ALU.mult)
    nc.vector.scalar_tensor_tensor(trow[:], s2[:], 1.0 / D, trow[:], ALU.mult, ALU.subtract)
    # inv_std = exp(-0.5*ln(var+eps)) : avoids banned Rsqrt and the very slow
    # 1-partition DVE reciprocal (3.3us); two fast ACT LUT ops instead.
    nc.scalar.activation(trow[:], trow[:], AF.Ln, bias=pg.eps[:])
    inv_r = pg.rpool.tile([1, T], rdt, tag="inv_r")
    nc.scalar.activation(inv_r[:], trow[:], AF.Exp, scale=-0.5)
    nmu_r = pg.rpool.tile([1, T], rdt, tag="nmu_r")
    nc.vector.scalar_tensor_tensor(nmu_r[:], mu[:], -1.0, inv_r[:], ALU.mult, ALU.mult)
    invb = pg.stps.tile([128, T], F32, tag="invb")
    nc.tensor.matmul(invb[:], ones_row[0:1, :], inv_r[:], start=True, stop=True)
    nmub = pg.stps.tile([128, T], F32, tag="nmub")
    nc.tensor.matmul(nmub[:], ones_row[0:1, :], nmu_r[:], start=True, stop=True)
    for c in range(Cf):
        xb = x_wide[:, c * T:(c + 1) * T]
        tmp = pg.spool.tile([128, T], F32, tag="lntmp")
        nc.vector.tensor_tensor(tmp[:], xb, invb[:], ALU.mult)
        nc.vector.tensor_tensor(tmp[:], tmp[:], nmub[:], ALU.add)
        emit_act(pg, xb, tmp[:], act)   # in-place write back into x_wide


def htile(pg, Cout, dt):
    pool = pg.p1a if Cout >= 16 or Cout <= 4 else pg.p1b
    return pool.tile([128, Cout * T], dt, tag=f"h{Cout}", name=f"h{Cout}")


def emit_chain(pg, x_wide, spec, w_aps, dt, final_consume=None):
    """Run a lin/ln/act spec chain feature-major. Returns the final wide tile,
    or None if the chain ends with a bare lin consumed by final_consume."""
    nc = pg.nc
    h = x_wide
    wi = 0
    i = 0
    n = len(spec)
    while i < n:
        op = spec[i]
        assert op[0] == "lin", op
        fi, fo = op[1], op[2]
        if i == n - 1:  # final linear
            emit_linear(pg, h, fi, fo, w_aps[wi], dt, final_consume)
            return None
        nxt = spec[i + 1][0]
        nxt2 = spec[i + 2][0] if i + 2 < n else None
        Cout = fo // 128
        hn = htile(pg, Cout, dt)
        if nxt == "ln":
            act = nxt2 if nxt2 in ACT_NAMES else None

            def cp(ps, co, mw, hn=hn):
                nc.scalar.copy(hn[:, co * T:(co + 1) * T], ps[:])
            emit_linear(pg, h, fi, fo, w_aps[wi], dt, cp)
            emit_ln_act(pg, hn, Cout, act, dt)
            i += 3 if act else 2
        elif nxt in ACT_NAMES:
            def ac(ps, co, mw, hn=hn, nxt=nxt):
                emit_act(pg, hn[:, co * T:(co + 1) * T], ps[:], nxt)
            emit_linear(pg, h, fi, fo, w_aps[wi], dt, ac)
            i += 2
        else:  # lin follows lin directly
            def cc(ps, co, mw, hn=hn):
                nc.scalar.copy(hn[:, co * T:(co + 1) * T], ps[:])
            emit_linear(pg, h, fi, fo, w_aps[wi], dt, cc)
            i += 1
        h = hn
        wi += 1
    return h


def emit_routing(pg, logits_sb):
    """logits_sb [E, T] f32 -> top-2 softmax routing weights [E, T] f32r."""
    nc = pg.nc
    ident = pg.C["ident"]
    wps = pg.stps.tile([E, T], F32, tag="invb")
    for c in range(T // 128):
        tp = pg.mmps.tile([128, E], F32, tag="mm")
        nc.tensor.transpose(tp[:], logits_sb[:, c * 128:(c + 1) * 128], ident[0:E, 0:E])
        ltok = pg.rpool.tile([128, E], F32, tag="ltok")
        nc.scalar.copy(ltok[:], tp[:])
        v1 = pg.rpool.tile([128, 1], F32, tag="rt_v1")
        nc.vector.tensor_reduce(v1[:], ltok[:], mybir.AxisListType.X, ALU.max)
        eq1 = pg.rpool.tile([128, E], F32, tag="rt_eq1")
        nc.vector.tensor_scalar(eq1[:], ltok[:], v1[:], None, ALU.is_equal)
        msk = pg.rpool.tile([128, E], F32, tag="rt_msk")
        nc.vector.scalar_tensor_tensor(msk[:], eq1[:], -1e30, ltok[:], ALU.mult, ALU.add)
        v2 = pg.rpool.tile([128, 1], F32, tag="rt_v2")
        nc.vector.tensor_reduce(v2[:], msk[:], mybir.AxisListType.X, ALU.max)
        eq2 = pg.rpool.tile([128, E], F32, tag="rt_eq2")
        nc.vector.tensor_scalar(eq2[:], msk[:], v2[:], None, ALU.is_equal)
        d = pg.rpool.tile([128, 1], F32, tag="rt_d")
        nc.vector.tensor_tensor(d[:], v2[:], v1[:], ALU.subtract)
        ex = pg.rpool.tile([128, 1], F32, tag="rt_ex")
        nc.scalar.activation(ex[:], d[:], AF.Exp)
        den = pg.rpool.tile([128, 1], F32, tag="rt_den")
        nc.vector.tensor_scalar(den[:], ex[:], 1.0, None, ALU.add)
        w1 = pg.rpool.tile([128, 1], F32, tag="rt_w1")
        nc.vector.reciprocal(w1[:], den[:])
        w2 = pg.rpool.tile([128, 1], F32, tag="rt_w2")
        nc.vector.tensor_tensor(w2[:], ex[:], w1[:], ALU.mult)
        wt = pg.rpool.tile([128, E], F32, tag="rt_wt")
        nc.vector.tensor_scalar(wt[:], eq1[:], w1[:], None, ALU.mult)
        wt2 = pg.rpool.tile([128, E], F32, tag="rt_wt2")
        nc.vector.tensor_scalar(wt2[:], eq2[:], w2[:], None, ALU.mult)
        wt3 = pg.rpool.tile([128, E], F32, tag="rt_wt3")
        nc.vector.tensor_tensor(wt3[:], wt[:], wt2[:], ALU.add)
        nc.tensor.transpose(wps[0:E, c * 128:(c + 1) * 128], wt3[:], ident[:])
    wf = pg.p1b.tile([E, T], F32R, tag="wfull")
    nc.scalar.copy(wf[:], wps[:])
    return wf


def emit_modality(pg, dram, m, moe_out):
    """Gate + routing + dense experts for one modality. Writes moe_out
    ([128, 2T] f32r = the weighted top-2 mixture, feature-major P=256)."""
    nc = pg.nc
    ident = pg.C["ident"]

    # load + transpose input embeddings to feature-major, f32 and f32r copies
    xT_f = pg.p1a.tile([128, (G // 128) * T], F32, tag="xT_f")
    xT_r = pg.p1a.tile([128, (G // 128) * T], F32R, tag="xT_r")
    for tc4 in range(T // 128):
        xin = pg.spool.tile([128, G], F32, tag="xin")
        nc.sync.dma_start(xin[:], dram[f"x_{m}"][tc4 * 128:(tc4 + 1) * 128, :])
        for fb in range(G // 128):
            tp = pg.mmps.tile([128, 128], F32, tag="mm")
            nc.tensor.transpose(tp[:], xin[:, fb * 128:(fb + 1) * 128], ident[:])
            dst = slice(fb * T + tc4 * 128, fb * T + (tc4 + 1) * 128)
            nc.scalar.copy(xT_f[:, dst], tp[:])
            nc.vector.tensor_copy(xT_r[:, dst], tp[:])

    # gate (fp32) -> logits [E, T]
    logits_sb = pg.p1b.tile([E, T], F32, tag="logits")

    def fc_logits(ps, co, mw):
        nc.scalar.copy(logits_sb[:], ps[:])
    emit_chain(pg, xT_f, GATE_SPEC,
               [dram[f"{m}_g_w0"], dram[f"{m}_g_w1"]], F32, fc_logits)
    wf = emit_routing(pg, logits_sb)

    # dense experts, weighted accumulation into macc
    macc = pg.p1a.tile([128, 2 * T], F32, tag="macc")
    w_iter = iter(lin_names(m))
    for e in range(E):
        spec = MOD_SPECS[m][e % 4]
        w_aps = [dram[next(w_iter)[0]] for op in spec if op[0] == "lin"]
        wb_ps = pg.mmps.tile([128, T], F32, tag="mm")
        nc.tensor.matmul(wb_ps[:], pg.C["esel"][:, e * 128:(e + 1) * 128], wf[:],
                         start=True, stop=True)
        wb = pg.spool.tile([128, T], F32, tag="wb")
        nc.scalar.copy(wb[:], wb_ps[:])

        def fc(ps, co, mw, e=e, wb=wb):
            blk = macc[:, co * T:(co + 1) * T]
            if e == 0:
                nc.vector.scalar_tensor_tensor(blk, ps[:], 1.0, wb[:],
                                               ALU.mult, ALU.mult)
            else:
                tw = pg.spool.tile([128, T], F32, tag="tw")
                nc.vector.scalar_tensor_tensor(tw[:], ps[:], 1.0, wb[:],
                                               ALU.mult, ALU.mult)
                nc.vector.tensor_tensor(blk, blk, tw[:], ALU.add)
        emit_chain(pg, xT_r, spec, w_aps, F32R, fc)
    nc.scalar.copy(moe_out[:], macc[:])


def emit_attention_and_head(pg, dram, moe_r, out_ap):
    nc = pg.nc
    ident = pg.C["ident"]

    # qkv per position: [128, 6T] f32r (q=blocks 0-1, k=2-3, v=4-5)
    qkv = []
    for pi, m in enumerate(MODS):
        qt = pg.p1a.tile([128, 6 * T], F32R, tag=f"qkv{pi}")

        def cq(ps, co, mw, qt=qt):
            nc.scalar.copy(qt[:, co * T:(co + 1) * T], ps[:])
        emit_linear(pg, moe_r[m], P, 3 * P, dram["attn_w_in"], F32R, cq)
        qkv.append(qt)

    combined = pg.p1a.tile([128, 6 * T], F32R, tag="comb")
    sc_tags = ("s1", "s2", "nmub")
    for qp in range(3):
        # scores [4, T] per kp  (scaled by 1/sqrt(HD)=0.125)
        scs = []
        for kp in range(3):
            scp = pg.stps.tile([4, T], F32, tag=sc_tags[kp])
            for c in range(2):
                prod = pg.spool.tile([128, T], F32R, tag="prod")
                nc.vector.tensor_tensor(prod[:], qkv[qp][:, c * T:(c + 1) * T],
                                        qkv[kp][:, (2 + c) * T:(3 + c) * T], ALU.mult)
                nc.tensor.matmul(scp[:], pg.C[f"ind{c}"][:], prod[:],
                                 start=(c == 0), stop=(c == 1))
            sc = pg.rpool.tile([4, T], F32, tag=f"sc{kp}")
            nc.scalar.activation(sc[:], scp[:], AF.Copy, scale=0.125)
            scs.append(sc)
        # softmax over kp
        mx = pg.rpool.tile([4, T], F32, tag="mx")
        nc.vector.tensor_tensor(mx[:], scs[0][:], scs[1][:], ALU.max)
        nc.vector.tensor_tensor(mx[:], mx[:], scs[2][:], ALU.max)
        es = []
        for kp in range(3):
            ek = scs[kp]
            nc.vector.tensor_tensor(ek[:], ek[:], mx[:], ALU.subtract)
            nc.scalar.activation(ek[:], ek[:], AF.Exp)
            es.append(ek)
        den = pg.rpool.tile([4, T], F32, tag="den4")
        nc.vector.tensor_tensor(den[:], es[0][:], es[1][:], ALU.add)
        nc.vector.tensor_tensor(den[:], den[:], es[2][:], ALU.add)
        # 1/den = exp(-ln(den)) on ACT (DVE reciprocal is ~6.5ns/elem serial)
        nc.scalar.activation(den[:], den[:], AF.Ln)
        rec = den
        nc.scalar.activation(rec[:], den[:], AF.Exp, scale=-1.0)
        # out: o = sum_kp bcast(att_kp) * v_kp   -> o_r [128, 2T] f32r
        oacc = pg.p1b.tile([128, 2 * T], F32, tag="oacc")
        o_r = pg.p1b.tile([128, 2 * T], F32R, tag="o_r")
        for kp in range(3):
            att = pg.rpool.tile([4, T], F32R, tag="attk")
            nc.vector.tensor_tensor(att[:], es[kp][:], rec[:], ALU.mult)
            for c in range(2):
                bc = pg.mmps.tile([128, T], F32, tag="mm")
                nc.tensor.matmul(bc[:], pg.C[f"bind{c}"][:], att[:],
                                 start=True, stop=True)
                vblk = qkv[kp][:, (4 + c) * T:(5 + c) * T]
                if kp == 0:
                    nc.vector.scalar_tensor_tensor(
                        oacc[:, c * T:(c + 1) * T], vblk, 1.0, bc[:], ALU.mult, ALU.mult)
                elif kp == 1:
                    tv = pg.spool.tile([128, T], F32, tag="tv")
                    nc.vector.scalar_tensor_tensor(tv[:], vblk, 1.0, bc[:],
                                                   ALU.mult, ALU.mult)
                    nc.vector.tensor_tensor(oacc[:, c * T:(c + 1) * T],
                                            oacc[:, c * T:(c + 1) * T], tv[:], ALU.add)
                else:
                    tv = pg.spool.tile([128, T], F32, tag="tv")
                    nc.vector.scalar_tensor_tensor(tv[:], vblk, 1.0, bc[:],
                                                   ALU.mult, ALU.mult)
                    nc.vector.tensor_tensor(o_r[:, c * T:(c + 1) * T],
                                            oacc[:, c * T:(c + 1) * T], tv[:], ALU.add)

        def co_out(ps, co, mw, qp=qp):
            nc.scalar.copy(combined[:, (qp * 2 + co) * T:(qp * 2 + co + 1) * T], ps[:])
        emit_linear(pg, o_r, P, P, dram["attn_w_out"], F32R, co_out)

    # inter MLP -> sigmoid gate; combined *= gate
    gate6 = emit_chain(pg, combined, INTER_SPEC,
                       [dram["inter_w0"], dram["inter_w1"]], F32R)
    for c in range(6):
        nc.vector.tensor_tensor(combined[:, c * T:(c + 1) * T],
                                combined[:, c * T:(c + 1) * T],
                                gate6[:, c * T:(c + 1) * T], ALU.mult)

    # out MLP -> final [F, T] then transpose to [T, F] and DMA out
    fin = pg.p1a.tile([128, 6 * T], F32, tag="fin")

    def fc_fin(ps, co, mw):
        nc.scalar.copy(fin[:, co * T:(co + 1) * T], ps[:])
    emit_chain(pg, combined, OUT_SPEC, [dram["out_w0"], dram["out_w1"]], F32R, fc_fin)

    for tc4 in range(T // 128):
        ot = pg.p1b.tile([128, F], F32, tag="otok")
        for fb in range(F // 128):
            tp = pg.mmps.tile([128, 128], F32, tag="mm")
            nc.tensor.transpose(
                tp[:], fin[:, fb * T + tc4 * 128:fb * T + (tc4 + 1) * 128], ident[:])
            nc.vector.tensor_copy(ot[:, fb * 128:(fb + 1) * 128], tp[:])
        nc.sync.dma_start(out_ap[tc4 * 128:(tc4 + 1) * 128, :], ot[:])


def emit_all(pg, dram, out_ap):
    nc, tc, ctx = pg.nc, pg.tc, pg.ctx
    cpool = ctx.enter_context(tc.tile_pool(name="consts", bufs=1))
    gpool = ctx.enter_context(tc.tile_pool(name="glob", bufs=1))
    pg.wpool = ctx.enter_context(tc.tile_pool(name="wkb", bufs=8))
    pg.spool = ctx.enter_context(tc.tile_pool(name="small", bufs=2))
    pg.rpool = ctx.enter_context(tc.tile_pool(name="rows", bufs=1))
    pg.mmps = ctx.enter_context(tc.tile_pool(name="mmps", bufs=4, space="PSUM"))
    pg.stps = ctx.enter_context(tc.tile_pool(name="stps", bufs=1, space="PSUM"))

    pg.C = {}
    for nm in ("ident", "ones_col_f", "ones_col_r", "ones_row_f", "ones_row_r",
               "esel", "ind0", "ind1", "bind0", "bind1"):
        t = cpool.tile(list(dram[nm].shape), dram[nm].dtype, tag=nm, name=f"c_{nm}")
        nc.sync.dma_start(t[:], dram[nm][:])
        pg.C[nm] = t
    pg.eps = cpool.tile([1, 1], F32, tag="eps")
    nc.vector.memset(pg.eps[:], 1e-5)

    moe_r = {}
    for m in MODS:
        moe_r[m] = gpool.tile([128, 2 * T], F32R, tag=f"moe_{m}", name=f"moe_{m}")

    with tc.tile_pool(name="ph1a", bufs=1) as p1a, \
         tc.tile_pool(name="ph1b", bufs=2) as p1b:
        pg.p1a, pg.p1b = p1a, p1b
        for m in MODS:
            emit_modality(pg, dram, m, moe_r[m])

    with tc.tile_pool(name="ph2a", bufs=1) as p2a, \
         tc.tile_pool(name="ph2b", bufs=2) as p2b:
        pg.p1a, pg.p1b = p2a, p2b
        emit_attention_and_head(pg, dram, moe_r, out_ap)


def build_program():
    nc = bacc.Bacc("TRN2", target_bir_lowering=False, debug=False,
                   num_devices=NCORES)
    dram = {}

    def din(name, shape, dt):
        dram[name] = nc.dram_tensor(name, list(shape), dt, kind="ExternalInput").ap()

    for m in MODS:
        din(f"x_{m}", (T, G), F32)
        for nm, fi, fo in lin_names(m):
            din(nm, (fi, fo), F32R)
        din(f"{m}_g_w0", (G, H // 2), F32)
        din(f"{m}_g_w1", (H // 2, E), F32)
    din("attn_w_in", (P, 3 * P), F32R)
    din("attn_w_out", (P, P), F32R)
    for i in range(2):
        din(f"inter_w{i}", (F, F), F32R)
        din(f"out_w{i}", (F, F), F32R)
    din("ident", (128, 128), F32)
    din("ones_col_f", (128, 1), F32)
    din("ones_col_r", (128, 1), F32R)
    din("ones_row_f", (1, 128), F32)
    din("ones_row_r", (1, 128), F32R)
    din("esel", (E, E * 128), F32R)
    din("ind0", (128, 4), F32R)
    din("ind1", (128, 4), F32R)
    din("bind0", (4, 128), F32R)
    din("bind1", (4, 128), F32R)
    out_ap = nc.dram_tensor("out", [T, F], F32, kind="ExternalOutput").ap()

    pg = Prog()
    with tile.TileContext(nc) as tc, ExitStack() as ctx:
        pg.nc, pg.tc, pg.ctx = nc, tc, ctx
        emit_all(pg, dram, out_ap)
    nc.compile()
    return nc


# --------------------------------------------------------------------------
# host side
# --------------------------------------------------------------------------

_NC = None
_TRACE = False
_LAST_EXEC_NS = None


def _consts():
    c = {}
    c["ident"] = np.eye(128, dtype=np.float32)
    c["ones_col_f"] = np.ones((128, 1), np.float32)
    c["ones_col_r"] = np.ones((128, 1), np.float32)
    c["ones_row_f"] = np.ones((1, 128), np.float32)
    c["ones_row_r"] = np.ones((1, 128), np.float32)
    esel = np.zeros((E, E * 128), np.float32)
    for e in range(E):
        esel[e, e * 128:(e + 1) * 128] = 1.0
    c["esel"] = esel
    for ci in range(2):
        ind = np.zeros((128, 4), np.float32)
        ind[0:64, 2 * ci] = 1.0
        ind[64:128, 2 * ci + 1] = 1.0
        c[f"ind{ci}"] = ind
        c[f"bind{ci}"] = ind.T.copy()
    return c


def _flatten_params(params):
    """-> dict of np arrays keyed by dram tensor names; asserts the zero/unit
    structure of biases and LN params this kernel relies on."""
    out = {}

    def getw(plist, spec, prefix):
        li = 0
        pi = 0
        for op in spec:
            if op[0] == "lin":
                w, b = plist[pi]
                assert not np.any(np.asarray(b)), f"nonzero bias at {prefix}_w{li}"
                out[f"{prefix}_w{li}"] = np.asarray(w, np.float32)
                li += 1
                pi += 1
            elif op[0] == "ln":
                g, b = plist[pi]
                assert np.all(np.asarray(g) == 1.0) and not np.any(np.asarray(b)), \
                    f"non-identity LN at {prefix}"
                pi += 1

    for m in MODS:
        for e in range(E):
            getw(params[m][e], MOD_SPECS[m][e % 4], f"{m}_e{e}")
        gl = params[f"{m}_gate"]
        getw(gl, GATE_SPEC, f"{m}_g")
    ap = params["attn"]
    assert not np.any(np.asarray(ap["b_in"])) and not np.any(np.asarray(ap["b_out"]))
    out["attn_w_in"] = np.asarray(ap["w_in"], np.float32)
    out["attn_w_out"] = np.asarray(ap["w_out"], np.float32)
    getw(params["inter"], INTER_SPEC, "inter")
    getw(params["out"], OUT_SPEC, "out")
    return out


def kernel(ast_emb, pdg_emb, cfg_emb, params):
    global _NC, _LAST_EXEC_NS
    if _NC is None:
        _NC = build_program()
    flat = _flatten_params(params)
    # round the f32r weights on host (HW truncates to 20 bits on read)
    shared = {}
    f32_keys = {f"{m}_g_w{i}" for m in MODS for i in range(2)}
    for k, v in flat.items():
        shared[k] = v if k in f32_keys else round_f32r(v)
    shared.update(_consts())

    embs = {"ast": np.asarray(ast_emb, np.float32),
            "pdg": np.asarray(pdg_emb, np.float32),
            "cfg": np.asarray(cfg_emb, np.float32)}
    in_maps = []
    for core in range(NCORES):
        im = dict(shared)
        for m in MODS:
            im[f"x_{m}"] = embs[m][core * T:(core + 1) * T]
        in_maps.append(im)

    res = run_bass_kernel_spmd(_NC, in_maps, core_ids=list(range(NCORES)),
                               trace=_TRACE)
    _LAST_EXEC_NS = res.exec_time_ns
    return np.concatenate([res.results[c]["out"] for c in range(NCORES)], axis=0)


# revision 11
# speedup vs baseline: 1.2985x; 1.0001x over previous
"""Trainium2 Bass kernel for nn_EnhancedVulnerabilityDetector (moe_routing).

Sharding: data-parallel over batch, 8 cores x 512 tokens each.
Layout: feature-major ([feature_partitions, 512 tokens]) everywhere, so every
linear is a PE matmul with the weight stationary and the token block moving
(N=512, one PSUM bank). Experts are computed densely (all 8) and weighted by
exact top-2 routing weights. Heavy matmuls run in fp32r (fp32 rounded to 11
mantissa bits -> full PE speed); the gate path runs in plain fp32 so the top-2
selection matches the fp32 reference. Routing top-2 itself is done token-major
via PE transposes so reductions are free-axis / per-partition-scalar ops.
"""
import sys
sys.path.insert(0, "/opt/trn_rl_repo")
from contextlib import ExitStack
from functools import partial

import numpy as np

import concourse.tile as tile
from concourse import bacc, mybir
from concourse.bass_utils import run_bass_kernel_spmd

F32 = mybir.dt.float32
F32R = mybir.dt.float32r
AF = mybir.ActivationFunctionType
ALU = mybir.AluOpType

# ---- model dims (hardcoded from the problem spec) ----
G, H, P, F, E, B = 512, 1024, 256, 768, 8, 4096
NCORES = 8
T = B // NCORES          # 512 tokens per core

AST_SPECS = [
    [("lin", G, H), ("ln", H), ("gelu",), ("lin", H, H), ("ln", H), ("gelu",), ("lin", H, P)],
    [("lin", G, H), ("ln", H), ("relu",), ("lin", H, H), ("lin", H, H), ("lin", H, P)],
    [("lin", G, H), ("ln", H), ("elu",), ("lin", H, H), ("tanh",), ("lin", H, P)],
    [("lin", G, H // 2), ("ln", H // 2), ("gelu",), ("lin", H // 2, P)],
]
PDG_SPECS = [
    [("lin", G, H), ("ln", H), ("gelu",), ("lin", H, H), ("ln", H), ("gelu",), ("lin", H, H), ("gelu",), ("lin", H, P)],
    [("lin", G, H), ("ln", H), ("sigmoid",), ("lin", H, H), ("relu",), ("lin", H, P)],
    [("lin", G, 2 * H), ("ln", 2 * H), ("relu",), ("lin", 2 * H, H), ("ln", H), ("gelu",), ("lin", H, P)],
    [("lin", G, H // 2), ("ln", H // 2), ("leaky",), ("lin", H // 2, P)],
]
CFG_SPECS = [
    [("lin", G, H), ("ln", H), ("gelu",), ("lin", H, H), ("ln", H), ("gelu",), ("lin", H, H), ("ln", H), ("gelu",), ("lin", H, P)],
    [("lin", G, H), ("ln", H), ("tanh",), ("lin", H, H), ("ln", H), ("relu",), ("lin", H, P)],
    [("lin", G, H), ("ln", H), ("gelu",), ("lin", H, 2 * H), ("sigmoid",), ("lin", 2 * H, H), ("ln", H), ("gelu",), ("lin", H, P)],
    [("lin", G, H // 2), ("relu",), ("lin", H // 2, P)],
]
GATE_SPEC = [("lin", G, H // 2), ("ln", H // 2), ("gelu",), ("lin", H // 2, E)]
INTER_SPEC = [("lin", F, F), ("ln", F), ("gelu",), ("lin", F, F), ("ln", F), ("sigmoid",)]
OUT_SPEC = [("lin", F, F), ("ln", F), ("gelu",), ("lin", F, F)]
MODS = ("ast", "pdg", "cfg")
MOD_SPECS = {"ast": AST_SPECS, "pdg": PDG_SPECS, "cfg": CFG_SPECS}
ACT_FUNCS = {"gelu": AF.Gelu, "relu": AF.Relu, "tanh": AF.Tanh,
             "sigmoid": AF.Sigmoid, "leaky": AF.Lrelu}
ACT_NAMES = set(ACT_FUNCS) | {"elu"}


def round_f32r(a):
    b = np.ascontiguousarray(np.asarray(a, np.float32)).view(np.uint32)
    r = (b + 0x7FF + ((b >> 12) & 1)) & np.uint32(0xFFFFF000)
    return r.view(np.float32).copy()


def lin_names(mod):
    out = []
    for e in range(E):
        li = 0
        for op in MOD_SPECS[mod][e % 4]:
            if op[0] == "lin":
                out.append((f"{mod}_e{e}_w{li}", op[1], op[2]))
                li += 1
    return out


class Prog:
    pass


# --------------------------------------------------------------------------
# emitters
# --------------------------------------------------------------------------

def emit_linear(pg, x_wide, fi, fo, w_ap, dt, consume, wtag="wkb"):
    """y = W.T @ x feature-major. x_wide: [128, (fi/128)*T]. For each output
    block co (width mw<=128) calls consume(psum_tile, co, mw)."""
    nc = pg.nc
    Cin = fi // 128
    Cout = (fo + 127) // 128
    for og0 in range(0, Cout, 4):
        og1 = min(og0 + 4, Cout)
        blocks = []
        for co in range(og0, og1):
            mw = min(128, fo - co * 128)
            blocks.append((co, mw, pg.mmps.tile([mw, T], F32, tag="mm", name="mmblk")))
        f0 = og0 * 128
        f1 = min(og1 * 128, fo)
        for ci in range(Cin):
            wkb = pg.wpool.tile([128, f1 - f0], dt, tag=wtag, name="wkb")
            nc.sync.dma_start(wkb[:, :], w_ap[ci * 128:(ci + 1) * 128, f0:f1])
            for co, mw, ps in blocks:
                nc.tensor.matmul(
                    ps[:], wkb[:, co * 128 - f0:co * 128 - f0 + mw],
                    x_wide[:, ci * T:(ci + 1) * T],
                    start=(ci == 0), stop=(ci == Cin - 1))
        for co, mw, ps in blocks:
            consume(ps, co, mw)


def emit_act(pg, dst, src, act, extra_scale=None):
    """dst = act(src) on the scalar engine (src may be PSUM)."""
    nc = pg.nc
    kw = {}
    if extra_scale is not None:
        kw["scale"] = extra_scale
    if act is None:
        nc.scalar.activation(dst, src, AF.Copy, **kw)
    elif act == "elu":
        emit_elu(pg, src, dst)
    elif act == "leaky":
        nc.scalar.activation(dst, src, AF.Lrelu, alpha=0.01, **kw)
    else:
        nc.scalar.activation(dst, src, ACT_FUNCS[act], **kw)


def emit_elu(pg, src, dst):
    """dst = elu(src) = relu(src) + exp(min(src,0)) - 1."""
    nc = pg.nc
    t1 = pg.spool.tile([128, T], F32, tag="elu1")
    nc.vector.tensor_scalar(t1[:], src, 0.0, None, ALU.min)
    t2 = pg.spool.tile([128, T], F32, tag="elu2")
    nc.scalar.activation(t2[:], t1[:], AF.Exp)
    t3 = pg.spool.tile([128, T], F32, tag="elu3")
    nc.scalar.activation(t3[:], src, AF.Relu)
    nc.vector.tensor_tensor(t2[:], t2[:], t3[:], ALU.add)
    nc.vector.tensor_scalar(dst, t2[:], 1.0, None, ALU.subtract)


def emit_ln_act(pg, x_wide, Cf, act, rdt):
    """In-place LayerNorm over Cf*128 features + activation on x_wide."""
    nc = pg.nc
    ones_col = pg.C["ones_col_r" if rdt == F32R else "ones_col_f"]
    ones_row = pg.C["ones_row_r" if rdt == F32R else "ones_row_f"]
    D = float(Cf * 128)
    s1 = pg.stps.tile([1, T], F32, tag="s1")
    s2 = pg.stps.tile([1, T], F32, tag="s2")
    for c in range(Cf):
        xb = x_wide[:, c * T:(c + 1) * T]
        nc.tensor.matmul(s1[:], ones_col[:], xb, start=(c == 0), stop=(c == Cf - 1))
        sq = pg.spool.tile([128, T], rdt, tag="sq")
        nc.vector.tensor_tensor(sq[:], xb, xb, ALU.mult)
        nc.tensor.matmul(s2[:], ones_col[:], sq[:], start=(c == 0), stop=(c == Cf - 1))
    mu = pg.rpool.tile([1, T], F32, tag="mu")
    nc.vector.tensor_scalar(mu[:], s1[:], 1.0 / D, None, ALU.mult)
    trow = pg.rpool.tile([1, T], F32, tag="trow")
    nc.vector.tensor_tensor(trow[:], mu[:], mu[:], ALU.mult)
    nc.vector.scalar_tensor_tensor(trow[:], s2[:], 1.0 / D, trow[:], ALU.mult, ALU.subtract)
    # inv_std = exp(-0.5*ln(var+eps)) : avoids banned Rsqrt and the very slow
    # 1-partition DVE reciprocal (3.3us); two fast ACT LUT ops instead.
    nc.scalar.activation(trow[:], trow[:], AF.Ln, bias=pg.eps[:])
    inv_r = pg.rpool.tile([1, T], rdt, tag="inv_r")
    nc.scalar.activation(inv_r[:], trow[:], AF.Exp, scale=-0.5)
    nmu_r = pg.rpool.tile([1, T], rdt, tag="nmu_r")
    nc.vector.scalar_tensor_tensor(nmu_r[:], mu[:], -1.0, inv_r[:], ALU.mult, ALU.mult)
    invb = pg.stps.tile([128, T], F32, tag="invb")
    nc.tensor.matmul(invb[:], ones_row[0:1, :], inv_r[:], start=True, stop=True)
    nmub = pg.stps.tile([128, T], F32, tag="nmub")
    nc.tensor.matmul(nmub[:], ones_row[0:1, :], nmu_r[:], start=True, stop=True)
    for c in range(Cf):
        xb = x_wide[:, c * T:(c + 1) * T]
        tmp = pg.spool.tile([128, T], F32, tag="lntmp")
        nc.vector.tensor_tensor(tmp[:], xb, invb[:], ALU.mult)
        nc.vector.tensor_tensor(tmp[:], tmp[:], nmub[:], ALU.add)
        emit_act(pg, xb, tmp[:], act)   # in-place write back into x_wide


def htile(pg, Cout, dt):
    pool = pg.p1a if Cout >= 16 or Cout <= 4 else pg.p1b
    return pool.tile([128, Cout * T], dt, tag=f"h{Cout}", name=f"h{Cout}")


def emit_chain(pg, x_wide, spec, w_aps, dt, final_consume=None):
    """Run a lin/ln/act spec chain feature-major. Returns the final wide tile,
    or None if the chain ends with a bare lin consumed by final_consume."""
    nc = pg.nc
    h = x_wide
    wi = 0
    i = 0
    n = len(spec)
    while i < n:
        op = spec[i]
        assert op[0] == "lin", op
        fi, fo = op[1], op[2]
        wtag = "wkbF" if wi == 0 else "wkb"
        if i == n - 1:  # final linear
            emit_linear(pg, h, fi, fo, w_aps[wi], dt, final_consume, wtag=wtag)
            return None
        nxt = spec[i + 1][0]
        nxt2 = spec[i + 2][0] if i + 2 < n else None
        Cout = fo // 128
        hn = htile(pg, Cout, dt)
        if nxt == "ln":
            act = nxt2 if nxt2 in ACT_NAMES else None

            def cp(ps, co, mw, hn=hn):
                nc.scalar.copy(hn[:, co * T:(co + 1) * T], ps[:])
            emit_linear(pg, h, fi, fo, w_aps[wi], dt, cp, wtag=wtag)
            emit_ln_act(pg, hn, Cout, act, dt)
            i += 3 if act else 2
        elif nxt in ACT_NAMES:
            def ac(ps, co, mw, hn=hn, nxt=nxt):
                emit_act(pg, hn[:, co * T:(co + 1) * T], ps[:], nxt)
            emit_linear(pg, h, fi, fo, w_aps[wi], dt, ac, wtag=wtag)
            i += 2
        else:  # lin follows lin directly
            def cc(ps, co, mw, hn=hn):
                nc.scalar.copy(hn[:, co * T:(co + 1) * T], ps[:])
            emit_linear(pg, h, fi, fo, w_aps[wi], dt, cc, wtag=wtag)
            i += 1
        h = hn
        wi += 1
    return h


def emit_routing(pg, logits_sb):
    """logits_sb [E, T] f32 -> top-2 softmax routing weights [E, T] f32r."""
    nc = pg.nc
    ident = pg.C["ident"]
    wps = pg.stps.tile([E, T], F32, tag="invb")
    for c in range(T // 128):
        tp = pg.mmps.tile([128, E], F32, tag="mm")
        nc.tensor.transpose(tp[:], logits_sb[:, c * 128:(c + 1) * 128], ident[0:E, 0:E])
        ltok = pg.rpool.tile([128, E], F32, tag="ltok")
        nc.scalar.copy(ltok[:], tp[:])
        v1 = pg.rpool.tile([128, 1], F32, tag="rt_v1")
        nc.vector.tensor_reduce(v1[:], ltok[:], mybir.AxisListType.X, ALU.max)
        eq1 = pg.rpool.tile([128, E], F32, tag="rt_eq1")
        nc.vector.tensor_scalar(eq1[:], ltok[:], v1[:], None, ALU.is_equal)
        msk = pg.rpool.tile([128, E], F32, tag="rt_msk")
        nc.vector.scalar_tensor_tensor(msk[:], eq1[:], -1e30, ltok[:], ALU.mult, ALU.add)
        v2 = pg.rpool.tile([128, 1], F32, tag="rt_v2")
        nc.vector.tensor_reduce(v2[:], msk[:], mybir.AxisListType.X, ALU.max)
        eq2 = pg.rpool.tile([128, E], F32, tag="rt_eq2")
        nc.vector.tensor_scalar(eq2[:], msk[:], v2[:], None, ALU.is_equal)
        d = pg.rpool.tile([128, 1], F32, tag="rt_d")
        nc.vector.tensor_tensor(d[:], v2[:], v1[:], ALU.subtract)
        ex = pg.rpool.tile([128, 1], F32, tag="rt_ex")
        nc.scalar.activation(ex[:], d[:], AF.Exp)
        den = pg.rpool.tile([128, 1], F32, tag="rt_den")
        nc.vector.tensor_scalar(den[:], ex[:], 1.0, None, ALU.add)
        w1 = pg.rpool.tile([128, 1], F32, tag="rt_w1")
        nc.vector.reciprocal(w1[:], den[:])
        w2 = pg.rpool.tile([128, 1], F32, tag="rt_w2")
        nc.vector.tensor_tensor(w2[:], ex[:], w1[:], ALU.mult)
        wt = pg.rpool.tile([128, E], F32, tag="rt_wt")
        nc.vector.tensor_scalar(wt[:], eq1[:], w1[:], None, ALU.mult)
        wt2 = pg.rpool.tile([128, E], F32, tag="rt_wt2")
        nc.vector.tensor_scalar(wt2[:], eq2[:], w2[:], None, ALU.mult)
        wt3 = pg.rpool.tile([128, E], F32, tag="rt_wt3")
        nc.vector.tensor_tensor(wt3[:], wt[:], wt2[:], ALU.add)
        nc.tensor.transpose(wps[0:E, c * 128:(c + 1) * 128], wt3[:], ident[:])
    wf = pg.p1b.tile([E, T], F32R, tag="wfull")
    nc.scalar.copy(wf[:], wps[:])
    return wf


def emit_modality(pg, dram, m, moe_out):
    """Gate + routing + dense experts for one modality. Writes moe_out
    ([128, 2T] f32r = the weighted top-2 mixture, feature-major P=256)."""
    nc = pg.nc
    ident = pg.C["ident"]

    # load + transpose input embeddings to feature-major, f32 and f32r copies
    xT_f = pg.p1a.tile([128, (G // 128) * T], F32, tag="xT_f")
    xT_r = pg.p1a.tile([128, (G // 128) * T], F32R, tag="xT_r")
    for tc4 in range(T // 128):
        xin = pg.spool.tile([128, G], F32, tag="xin")
        nc.sync.dma_start(xin[:], dram[f"x_{m}"][tc4 * 128:(tc4 + 1) * 128, :])
        for fb in range(G // 128):
            tp = pg.mmps.tile([128, 128], F32, tag="mm")
            nc.tensor.transpose(tp[:], xin[:, fb * 128:(fb + 1) * 128], ident[:])
            dst = slice(fb * T + tc4 * 128, fb * T + (tc4 + 1) * 128)
            nc.scalar.copy(xT_f[:, dst], tp[:])
            nc.vector.tensor_copy(xT_r[:, dst], tp[:])

    # gate (fp32) -> logits [E, T]
    logits_sb = pg.p1b.tile([E, T], F32, tag="logits")

    def fc_logits(ps, co, mw):
        nc.scalar.copy(logits_sb[:], ps[:])
    emit_chain(pg, xT_f, GATE_SPEC,
               [dram[f"{m}_g_w0"], dram[f"{m}_g_w1"]], F32, fc_logits)
    wf = emit_routing(pg, logits_sb)

    # dense experts, weighted accumulation into macc
    macc = pg.p1a.tile([128, 2 * T], F32, tag="macc")
    w_iter = iter(lin_names(m))
    for e in range(E):
        spec = MOD_SPECS[m][e % 4]
        w_aps = [dram[next(w_iter)[0]] for op in spec if op[0] == "lin"]
        wb_ps = pg.mmps.tile([128, T], F32, tag="mm")
        nc.tensor.matmul(wb_ps[:], pg.C["esel"][:, e * 128:(e + 1) * 128], wf[:],
                         start=True, stop=True)
        wb = pg.spool.tile([128, T], F32, tag="wb")
        nc.scalar.copy(wb[:], wb_ps[:])

        def fc(ps, co, mw, e=e, wb=wb):
            blk = macc[:, co * T:(co + 1) * T]
            if e == 0:
                nc.vector.scalar_tensor_tensor(blk, ps[:], 1.0, wb[:],
                                               ALU.mult, ALU.mult)
            else:
                tw = pg.spool.tile([128, T], F32, tag="tw")
                nc.vector.scalar_tensor_tensor(tw[:], ps[:], 1.0, wb[:],
                                               ALU.mult, ALU.mult)
                nc.vector.tensor_tensor(blk, blk, tw[:], ALU.add)
        emit_chain(pg, xT_r, spec, w_aps, F32R, fc)
    nc.scalar.copy(moe_out[:], macc[:])


def emit_attention_and_head(pg, dram, moe_r, out_ap):
    nc = pg.nc
    ident = pg.C["ident"]

    # qkv per position: [128, 6T] f32r (q=blocks 0-1, k=2-3, v=4-5)
    qkv = []
    for pi, m in enumerate(MODS):
        qt = pg.p1a.tile([128, 6 * T], F32R, tag=f"qkv{pi}")

        def cq(ps, co, mw, qt=qt):
            nc.scalar.copy(qt[:, co * T:(co + 1) * T], ps[:])
        emit_linear(pg, moe_r[m], P, 3 * P, dram["attn_w_in"], F32R, cq)
        qkv.append(qt)

    combined = pg.p1a.tile([128, 6 * T], F32R, tag="comb")
    sc_tags = ("s1", "s2", "nmub")
    for qp in range(3):
        # scores [4, T] per kp  (scaled by 1/sqrt(HD)=0.125)
        scs = []
        for kp in range(3):
            scp = pg.stps.tile([4, T], F32, tag=sc_tags[kp])
            for c in range(2):
                prod = pg.spool.tile([128, T], F32R, tag="prod")
                nc.vector.tensor_tensor(prod[:], qkv[qp][:, c * T:(c + 1) * T],
                                        qkv[kp][:, (2 + c) * T:(3 + c) * T], ALU.mult)
                nc.tensor.matmul(scp[:], pg.C[f"ind{c}"][:], prod[:],
                                 start=(c == 0), stop=(c == 1))
            sc = pg.rpool.tile([4, T], F32, tag=f"sc{kp}")
            nc.scalar.activation(sc[:], scp[:], AF.Copy, scale=0.125)
            scs.append(sc)
        # softmax over kp
        mx = pg.rpool.tile([4, T], F32, tag="mx")
        nc.vector.tensor_tensor(mx[:], scs[0][:], scs[1][:], ALU.max)
        nc.vector.tensor_tensor(mx[:], mx[:], scs[2][:], ALU.max)
        es = []
        for kp in range(3):
            ek = scs[kp]
            nc.vector.tensor_tensor(ek[:], ek[:], mx[:], ALU.subtract)
            nc.scalar.activation(ek[:], ek[:], AF.Exp)
            es.append(ek)
        den = pg.rpool.tile([4, T], F32, tag="den4")
        nc.vector.tensor_tensor(den[:], es[0][:], es[1][:], ALU.add)
        nc.vector.tensor_tensor(den[:], den[:], es[2][:], ALU.add)
        # 1/den = exp(-ln(den)) on ACT (DVE reciprocal is ~6.5ns/elem serial)
        nc.scalar.activation(den[:], den[:], AF.Ln)
        rec = den
        nc.scalar.activation(rec[:], den[:], AF.Exp, scale=-1.0)
        # out: o = sum_kp bcast(att_kp) * v_kp   -> o_r [128, 2T] f32r
        oacc = pg.p1b.tile([128, 2 * T], F32, tag="oacc")
        o_r = pg.p1b.tile([128, 2 * T], F32R, tag="o_r")
        for kp in range(3):
            att = pg.rpool.tile([4, T], F32R, tag="attk")
            nc.vector.tensor_tensor(att[:], es[kp][:], rec[:], ALU.mult)
            for c in range(2):
                bc = pg.mmps.tile([128, T], F32, tag="mm")
                nc.tensor.matmul(bc[:], pg.C[f"bind{c}"][:], att[:],
                                 start=True, stop=True)
                vblk = qkv[kp][:, (4 + c) * T:(5 + c) * T]
                if kp == 0:
                    nc.vector.scalar_tensor_tensor(
                        oacc[:, c * T:(c + 1) * T], vblk, 1.0, bc[:], ALU.mult, ALU.mult)
                elif kp == 1:
                    tv = pg.spool.tile([128, T], F32, tag="tv")
                    nc.vector.scalar_tensor_tensor(tv[:], vblk, 1.0, bc[:],
                                                   ALU.mult, ALU.mult)
                    nc.vector.tensor_tensor(oacc[:, c * T:(c + 1) * T],
                                            oacc[:, c * T:(c + 1) * T], tv[:], ALU.add)
                else:
                    tv = pg.spool.tile([128, T], F32, tag="tv")
                    nc.vector.scalar_tensor_tensor(tv[:], vblk, 1.0, bc[:],
                                                   ALU.mult, ALU.mult)
                    nc.vector.tensor_tensor(o_r[:, c * T:(c + 1) * T],
                                            oacc[:, c * T:(c + 1) * T], tv[:], ALU.add)

        def co_out(ps, co, mw, qp=qp):
            nc.scalar.copy(combined[:, (qp * 2 + co) * T:(qp * 2 + co + 1) * T], ps[:])
        emit_linear(pg, o_r, P, P, dram["attn_w_out"], F32R, co_out)

    # inter MLP -> sigmoid gate; combined *= gate
    gate6 = emit_chain(pg, combined, INTER_SPEC,
                       [dram["inter_w0"], dram["inter_w1"]], F32R)
    for c in range(6):
        nc.vector.tensor_tensor(combined[:, c * T:(c + 1) * T],
                                combined[:, c * T:(c + 1) * T],
                                gate6[:, c * T:(c + 1) * T], ALU.mult)

    # out MLP -> final [F, T] then transpose to [T, F] and DMA out
    fin = pg.p1a.tile([128, 6 * T], F32, tag="fin")

    def fc_fin(ps, co, mw):
        nc.scalar.copy(fin[:, co * T:(co + 1) * T], ps[:])
    emit_chain(pg, combined, OUT_SPEC, [dram["out_w0"], dram["out_w1"]], F32R, fc_fin)

    for tc4 in range(T // 128):
        ot = pg.p1a.tile([128, F], F32, tag="otok")
        for fb in range(F // 128):
            tp = pg.mmps.tile([128, 128], F32, tag="mm")
            nc.tensor.transpose(
                tp[:], fin[:, fb * T + tc4 * 128:fb * T + (tc4 + 1) * 128], ident[:])
            nc.vector.tensor_copy(ot[:, fb * 128:(fb + 1) * 128], tp[:])
        nc.sync.dma_start(out_ap[tc4 * 128:(tc4 + 1) * 128, :], ot[:])


def emit_all(pg, dram, out_ap):
    nc, tc, ctx = pg.nc, pg.tc, pg.ctx
    cpool = ctx.enter_context(tc.tile_pool(name="consts", bufs=1))
    gpool = ctx.enter_context(tc.tile_pool(name="glob", bufs=1))
    pg.wpool = ctx.enter_context(tc.tile_pool(name="wkb", bufs=6))
    pg.spool = ctx.enter_context(tc.tile_pool(name="small", bufs=2))
    pg.rpool = ctx.enter_context(tc.tile_pool(name="rows", bufs=1))
    pg.mmps = ctx.enter_context(tc.tile_pool(name="mmps", bufs=4, space="PSUM"))
    pg.stps = ctx.enter_context(tc.tile_pool(name="stps", bufs=1, space="PSUM"))

    pg.C = {}
    for nm in ("ident", "ones_col_f", "ones_col_r", "ones_row_f", "ones_row_r",
               "esel", "ind0", "ind1", "bind0", "bind1"):
        t = cpool.tile(list(dram[nm].shape), dram[nm].dtype, tag=nm, name=f"c_{nm}")
        nc.sync.dma_start(t[:], dram[nm][:])
        pg.C[nm] = t
    pg.eps = cpool.tile([1, 1], F32, tag="eps")
    nc.vector.memset(pg.eps[:], 1e-5)

    moe_r = {}
    for m in MODS:
        moe_r[m] = gpool.tile([128, 2 * T], F32R, tag=f"moe_{m}", name=f"moe_{m}")

    with tc.tile_pool(name="ph1a", bufs=1) as p1a, \
         tc.tile_pool(name="ph1b", bufs=2) as p1b:
        pg.p1a, pg.p1b = p1a, p1b
        for m in MODS:
            emit_modality(pg, dram, m, moe_r[m])

    with tc.tile_pool(name="ph2a", bufs=1) as p2a, \
         tc.tile_pool(name="ph2b", bufs=2) as p2b:
        pg.p1a, pg.p1b = p2a, p2b
        emit_attention_and_head(pg, dram, moe_r, out_ap)


def build_program():
    nc = bacc.Bacc("TRN2", target_bir_lowering=False, debug=False,
                   num_devices=NCORES)
    dram = {}

    def din(name, shape, dt):
        dram[name] = nc.dram_tensor(name, list(shape), dt, kind="ExternalInput").ap()

    for m in MODS:
        din(f"x_{m}", (T, G), F32)
        for nm, fi, fo in lin_names(m):
            din(nm, (fi, fo), F32R)
        din(f"{m}_g_w0", (G, H // 2), F32)
        din(f"{m}_g_w1", (H // 2, E), F32)
    din("attn_w_in", (P, 3 * P), F32R)
    din("attn_w_out", (P, P), F32R)
    for i in range(2):
        din(f"inter_w{i}", (F, F), F32R)
        din(f"out_w{i}", (F, F), F32R)
    din("ident", (128, 128), F32)
    din("ones_col_f", (128, 1), F32)
    din("ones_col_r", (128, 1), F32R)
    din("ones_row_f", (1, 128), F32)
    din("ones_row_r", (1, 128), F32R)
    din("esel", (E, E * 128), F32R)
    din("ind0", (128, 4), F32R)
    din("ind1", (128, 4), F32R)
    din("bind0", (4, 128), F32R)
    din("bind1", (4, 128), F32R)
    out_ap = nc.dram_tensor("out", [T, F], F32, kind="ExternalOutput").ap()

    pg = Prog()
    with tile.TileContext(nc) as tc, ExitStack() as ctx:
        pg.nc, pg.tc, pg.ctx = nc, tc, ctx
        emit_all(pg, dram, out_ap)
    nc.compile()
    return nc


# --------------------------------------------------------------------------
# host side
# --------------------------------------------------------------------------

_NC = None
_TRACE = False
_LAST_EXEC_NS = None


def _consts():
    c = {}
    c["ident"] = np.eye(128, dtype=np.float32)
    c["ones_col_f"] = np.ones((128, 1), np.float32)
    c["ones_col_r"] = np.ones((128, 1), np.float32)
    c["ones_row_f"] = np.ones((1, 128), np.float32)
    c["ones_row_r"] = np.ones((1, 128), np.float32)
    esel = np.zeros((E, E * 128), np.float32)
    for e in range(E):
        esel[e, e * 128:(e + 1) * 128] = 1.0
    c["esel"] = esel
    for ci in range(2):
        ind = np.zeros((128, 4), np.float32)
        ind[0:64, 2 * ci] = 1.0
        ind[64:128, 2 * ci + 1] = 1.0
        c[f"ind{ci}"] = ind
        c[f"bind{ci}"] = ind.T.copy()
    return c


def _flatten_params(params):
    """-> dict of np arrays keyed by dram tensor names; asserts the zero/unit
    structure of biases and LN params this kernel relies on."""
    out = {}

    def getw(plist, spec, prefix):
        li = 0
        pi = 0
        for op in spec:
            if op[0] == "lin":
                w, b = plist[pi]
                assert not np.any(np.asarray(b)), f"nonzero bias at {prefix}_w{li}"
                out[f"{prefix}_w{li}"] = np.asarray(w, np.float32)
                li += 1
                pi += 1
            elif op[0] == "ln":
                g, b = plist[pi]
                assert np.all(np.asarray(g) == 1.0) and not np.any(np.asarray(b)), \
                    f"non-identity LN at {prefix}"
                pi += 1

    for m in MODS:
        for e in range(E):
            getw(params[m][e], MOD_SPECS[m][e % 4], f"{m}_e{e}")
        gl = params[f"{m}_gate"]
        getw(gl, GATE_SPEC, f"{m}_g")
    ap = params["attn"]
    assert not np.any(np.asarray(ap["b_in"])) and not np.any(np.asarray(ap["b_out"]))
    out["attn_w_in"] = np.asarray(ap["w_in"], np.float32)
    out["attn_w_out"] = np.asarray(ap["w_out"], np.float32)
    getw(params["inter"], INTER_SPEC, "inter")
    getw(params["out"], OUT_SPEC, "out")
    return out


def kernel(ast_emb, pdg_emb, cfg_emb, params):
    global _NC, _LAST_EXEC_NS
    if _NC is None:
        _NC = build_program()
    flat = _flatten_params(params)
    # round the f32r weights on host (HW truncates to 20 bits on read)
    shared = {}
    f32_keys = {f"{m}_g_w{i}" for m in MODS for i in range(2)}
    for k, v in flat.items():
        shared[k] = v if k in f32_keys else round_f32r(v)
    shared.update(_consts())

    embs = {"ast": np.asarray(ast_emb, np.float32),
            "pdg": np.asarray(pdg_emb, np.float32),
            "cfg": np.asarray(cfg_emb, np.float32)}
    in_maps = []
    for core in range(NCORES):
        im = dict(shared)
        for m in MODS:
            im[f"x_{m}"] = embs[m][core * T:(core + 1) * T]
        in_maps.append(im)

    res = run_bass_kernel_spmd(_NC, in_maps, core_ids=list(range(NCORES)),
                               trace=_TRACE)
    _LAST_EXEC_NS = res.exec_time_ns
    return np.concatenate([res.results[c]["out"] for c in range(NCORES)], axis=0)
